# revision 1
# baseline (speedup 1.0000x reference)
"""Trainium2 Bass kernel for nn_Attention_39015482916872.

Multi-head attention (B=2, N=2048, C=1024, H=16, D=64) with RoPE,
tensor-parallel over (batch, heads) across 8 NeuronCores: core c handles
batch c//4 and heads 4*(c%4)..4*(c%4)+3. Each core computes its heads'
QKV projection, RoPE, attention, and a partial output projection; the
host sums the 4 partials per batch (Megatron-style column-parallel
w_proj) and adds b_proj.

Device-side design notes:
 - All matmuls in bf16 (f32 accumulation in PSUM).
 - x is cast f32->bf16 by a DMA (SWDGE cast) into internal DRAM, then
   DMA-transposed (XBAR) into SBUF so C lands on partitions.
 - RoPE pairing (even,odd) is conjugated by a fixed permutation into
   [real32 | imag32] halves per head (folded into w_qkv rows host-side;
   cancels in q.k since Q and K share it) so the DVE ops stay
   32/64-partition aligned.
 - Scores are computed transposed (n_k on partitions); softmax uses no
   max-subtraction (scores ~ N(0,1), exp cannot overflow) and the
   denominator comes from a 65th all-ones column appended to V; the
   division is applied to the small (D x n_q) PV output.
"""

import sys
from contextlib import ExitStack

import numpy as np

if "/opt/trn_rl_repo" not in sys.path:
    sys.path.insert(0, "/opt/trn_rl_repo")
try:
    import concourse.bass as bass
except ImportError:
    sys.path.insert(0, "/root/.axon_site/_ro/trn_rl_repo")
    import concourse.bass as bass
import concourse.tile as tile
from concourse import bacc, mybir
from concourse.bass_utils import run_bass_kernel_spmd

F32 = mybir.dt.float32
BF16 = mybir.dt.bfloat16
AF = mybir.ActivationFunctionType

B, N, C, H, D = 2, 2048, 1024, 16, 64
N_CORES = 8
CORES_PER_BATCH = N_CORES // B          # 4
HPC = H // CORES_PER_BATCH              # 4 heads per core


def build_attn_kernel(nc, tc, ctx, N=2048, C=1024, HPC=4, D=64, NQ_BLK=512, scale=None, phases=3, drive_mode='E', SB=2):
    P = 128
    KC = C // P
    QK_CHUNKS = 2 * HPC * D // P
    VF = HPC * D
    NB = N // NQ_BLK
    NKC = N // P
    NPC = N // P
    if scale is None:
        scale = D ** -0.5

    xb = nc.dram_tensor("xb", [N, C], F32, kind="ExternalInput").ap()
    wqkT = nc.dram_tensor("wqkT", [C, 2 * HPC * D], BF16, kind="ExternalInput").ap()
    wvT = nc.dram_tensor("wvT", [C, VF], BF16, kind="ExternalInput").ap()
    wpT = nc.dram_tensor("wpT", [VF, C], BF16, kind="ExternalInput").ap()
    cosT = nc.dram_tensor("cosT", [D // 2, N], BF16, kind="ExternalInput").ap()
    sinT = nc.dram_tensor("sinT", [D, N], BF16, kind="ExternalInput").ap()
    y = nc.dram_tensor("y", [N, C], F32, kind="ExternalOutput").ap()
    xbf = nc.dram_tensor("xbf_internal", [C // P, N, P], BF16).ap()

    persist = ctx.enter_context(tc.tile_pool(name="persist", bufs=1))
    psum_mm = ctx.enter_context(tc.tile_pool(name="psum_mm", bufs=2, space="PSUM"))
    psum_s = ctx.enter_context(tc.tile_pool(name="psum_s", bufs=2, space="PSUM"))
    psum_o = ctx.enter_context(tc.tile_pool(name="psum_o", bufs=1, space="PSUM"))
    rope_tmp = ctx.enter_context(tc.tile_pool(name="rope_tmp", bufs=3))
    exp_pool = ctx.enter_context(tc.tile_pool(name="exp_pool", bufs=4))
    norm_pool = ctx.enter_context(tc.tile_pool(name="norm_pool", bufs=2))
    y_pool = ctx.enter_context(tc.tile_pool(name="y_pool", bufs=4))

    NH = max(1, N // 1024)   # n-halves of 1024
    HW_ = N // NH            # half width
    xT = [[persist.tile([P, HW_], BF16, name=f"xT{k}_{h}", tag=f"xT{k}_{h}") for h in range(NH)] for k in range(KC)]

    def xT_slice(k, n0, w):
        h = n0 // HW_
        assert (n0 + w - 1) // HW_ == h
        return xT[k][h][:, n0 - h * HW_:n0 - h * HW_ + w]
    wqkT_sb = persist.tile([P, KC, 2 * HPC * D], BF16, tag="wqk")
    wvT_sb = persist.tile([P, KC, VF], BF16, tag="wv")
    wpT_sb = persist.tile([P, VF // P, C], BF16, tag="wp")
    cos_sb = persist.tile([P, N], BF16, tag="cos")
    sin_sb = persist.tile([P, N], BF16, tag="sin")
    qt = [[persist.tile([P, NQ_BLK], BF16, name=f"qt{i}_{j}", tag=f"qt{i}_{j}") for j in range(NB)] for i in range(QK_CHUNKS // 2)]
    kt = [[persist.tile([P, NQ_BLK], BF16, name=f"kt{i}_{j}", tag=f"kt{i}_{j}") for j in range(NB)] for i in range(QK_CHUNKS // 2)]
    vaug = [persist.tile([P, HPC, D + 1], BF16, name=f"va{j}", tag=f"va{j}") for j in range(NPC)]
    anorm = [persist.tile([P, N], BF16, name=f"an{i}", tag=f"an{i}") for i in range(VF // P)]

    # preload the exp activation table during the DMA prefix so the first
    # real softmax exp doesn't pay the ~2.7us ACT_TABLE_LOAD
    warm = persist.tile([1, 8], F32, tag="actwarm")
    nc.vector.memset(warm[:], 0.0)
    nc.scalar.activation(warm[:], warm[:], AF.Exp, scale=1.0)

    nc.sync.dma_start(wqkT_sb[:], wqkT.rearrange("(kc p) f -> p kc f", p=P))
    nc.sync.dma_start(wvT_sb[:], wvT.rearrange("(kc p) f -> p kc f", p=P))
    nc.sync.dma_start(wpT_sb[:], wpT.rearrange("(vc p) f -> p vc f", p=P))
    for g in range(P // (D // 2)):
        nc.sync.dma_start(cos_sb[g * (D // 2):(g + 1) * (D // 2), :], cosT[:, :])
    for g in range(P // D):
        nc.sync.dma_start(sin_sb[g * D:(g + 1) * D, :], sinT[:, :])
    for h in range(NH):
        n0, n1 = h * HW_, (h + 1) * HW_
        for k in range(KC):
            nc.gpsimd.dma_start(xbf[k, n0:n1, :], xb[n0:n1, k * P:(k + 1) * P])
            nc.sync.dma_start_transpose(xT[k][h][:], xbf[k, n0:n1, :])

    def v_chunk(j):
        pv = psum_mm.tile([P, VF], F32, name="pmm", tag="pmm", padded_shape=[P, 512])
        for k in range(KC):
            nc.tensor.matmul(
                pv[:],
                lhsT=xT_slice(k, j * P, P),
                rhs=wvT_sb[:, k, :],
                start=(k == 0),
                stop=(k == KC - 1),
            )
        nc.vector.memset(vaug[j][:, :, D], 1.0)
        nc.any.tensor_copy(vaug[j][:, :, 0:D], pv[:].rearrange("p (h d) -> p h d", d=D))

    if phases < 0.4:
        return
    if phases < 0.8:
        for j in range(NPC):
            v_chunk(j)
        return
    # Phase 1b: Q,K feat-major + RoPE (chunk feat layout: [h0r h0i h1r h1i])
    def rope_chunk(psum_c, dst, j):
        nb = j * NQ_BLK
        cs = cos_sb[:, nb:nb + NQ_BLK]
        sn = sin_sb[:, nb:nb + NQ_BLK]
        raw = rope_tmp.tile([P, NQ_BLK], BF16, tag="raw")
        nc.any.tensor_copy(raw[:], psum_c[:])
        tA = rope_tmp.tile([P, NQ_BLK], BF16, tag="tA")
        tB = rope_tmp.tile([P, NQ_BLK], BF16, tag="tB")
        nc.vector.tensor_mul(tA[:], raw[:], cs)
        # swapped sin product: out rows swap r<->i; the +/- sign is folded into
        # the sin table (rows [g*64:g*64+32] = +sin, [g*64+32:g*64+64] = -sin),
        # so DVE 2-input base partitions always match (walrus NCC_IBIR297).
        for g in range(2):
            b0 = 64 * g
            nc.vector.tensor_mul(tB[b0:b0 + 32, :], raw[b0 + 32:b0 + 64, :], sn[b0 + 32:b0 + 64, :])
            nc.vector.tensor_mul(tB[b0 + 32:b0 + 64, :], raw[b0:b0 + 32, :], sn[b0:b0 + 32, :])
        nc.vector.tensor_add(dst[:], tA[:], tB[:])

    def qk_block(qk, i, j):
        dst_t = qt[i][j] if qk == 0 else kt[i][j]
        fbase = (qk * (QK_CHUNKS // 2) + i) * P
        pqk = psum_mm.tile([P, NQ_BLK], F32, name="pmm", tag="pmm", padded_shape=[P, 512])
        for k in range(KC):
            nc.tensor.matmul(
                pqk[:],
                lhsT=wqkT_sb[:, k, fbase:fbase + P],
                rhs=xT_slice(k, j * NQ_BLK, NQ_BLK),
                start=(k == 0),
                stop=(k == KC - 1),
            )
        rope_chunk(pqk, dst_t[:], j)

    # Phase 2: attention per head, scores transposed (n_k on partitions).
    # psum_s batches SB nk-chunks so each exp covers SB*NQ_BLK elements
    # (amortizes the ~352-cycle ACT per-instruction overhead).
    def attention_pair_block(i, j):
        # Both heads of chunk i at n_q block j. The two MM1s at each kk use
        # disjoint PE row strips (rows 0-63 vs 64-127), issued back-to-back so
        # the hardware runs them concurrently; one exp covers both heads.
        h0, h1 = 2 * i, 2 * i + 1
        po0 = psum_o.tile([D + 1, NQ_BLK], F32, name="po0", tag="po0")
        po1 = psum_o.tile([D + 1, NQ_BLK], F32, name="po1", tag="po1")
        for kk in range(NKC):
            ps = psum_s.tile([P, 2, NQ_BLK], F32, tag="ps")
            kb, kc0 = divmod(kk * P, NQ_BLK)
            for g, h in ((0, h0), (1, h1)):
                hb = 64 * g
                nc.tensor.matmul(
                    ps[:, g, :],
                    lhsT=kt[i][kb][hb:hb + 64, kc0:kc0 + P],
                    rhs=qt[i][j][hb:hb + 64, :],
                    start=True,
                    stop=True,
                )
            es = exp_pool.tile([P, 2, NQ_BLK], BF16, tag="es")
            nc.scalar.activation(es[:], ps[:], AF.Exp, scale=float(scale))
            for g, h, po in ((0, h0, po0), (1, h1, po1)):
                nc.tensor.matmul(
                    po[:],
                    lhsT=vaug[kk][:, h, :],
                    rhs=es[:, g, :],
                    start=(kk == 0),
                    stop=(kk == NKC - 1),
                )
        for h, po in ((h0, po0), (h1, po1)):
            ot = norm_pool.tile([D + 1, NQ_BLK], F32, tag="ot")
            nc.vector.tensor_copy(ot[:], po[:])
            recip = norm_pool.tile([1, NQ_BLK], F32, tag="recip")
            nc.vector.reciprocal(recip[:], ot[D:D + 1, :])
            bcast = norm_pool.tile([64, NQ_BLK], F32, tag="bcast")
            nc.gpsimd.partition_broadcast(bcast[:], recip[:])
            dst = anorm[(h * D) // P]
            db = (h * D) % P
            nc.vector.tensor_mul(
                dst[db:db + D, j * NQ_BLK:(j + 1) * NQ_BLK], ot[0:D, :], bcast[:]
            )


    # Phase 3: partial output projection (natural layout, n on partitions)
    OB = min(512, C)
    NOB = C // OB
    def phase3_rows(j):
        for ob in range(NOB):
            py = psum_mm.tile([P, OB], F32, name="pmm", tag="pmm", padded_shape=[P, 512])
            for i in range(VF // P):
                nc.tensor.matmul(
                    py[:],
                    lhsT=anorm[i][:, j * P:(j + 1) * P],
                    rhs=wpT_sb[:, i, ob * OB:(ob + 1) * OB],
                    start=(i == 0),
                    stop=(i == VF // P - 1),
                )
            yt = y_pool.tile([P, OB], F32, tag="yt")
            nc.any.tensor_copy(yt[:], py[:])
            nc.sync.dma_start(y[j * P:(j + 1) * P, ob * OB:(ob + 1) * OB], yt[:])

    NPB = NQ_BLK // P       # 128-row chunks per nq block
    NCH = QK_CHUNKS // 2
    LH = NCH - 1
    if drive_mode in ("A", "D"):
        # V, then per chunk: K, Q, attention; phase3 interleaved (A) or last (D)
        for j in range(NPC):
            v_chunk(j)
        for i in range(NCH):
            for j in range(NB):
                qk_block(1, i, j)
            for j in range(NB):
                qk_block(0, i, j)
            if phases < 2:
                continue
            for j in range(NB):
                attention_pair_block(i, j)
                if phases >= 3 and i == LH and drive_mode == "A":
                    for jj in range(j * NPB, (j + 1) * NPB):
                        phase3_rows(jj)
        if phases >= 3 and drive_mode == "D":
            for jj in range(NPC):
                phase3_rows(jj)
    elif drive_mode == "B":
        # V, all QK chunks, then all attention
        for j in range(NPC):
            v_chunk(j)
        for i in range(NCH):
            for j in range(NB):
                qk_block(1, i, j)
            for j in range(NB):
                qk_block(0, i, j)
        if phases >= 2:
            for i in range(NCH):
                for j in range(NB):
                    attention_pair_block(i, j)
                    if phases >= 3 and i == LH:
                        for jj in range(j * NPB, (j + 1) * NPB):
                            phase3_rows(jj)
    elif drive_mode == "E":
        # earliest-exp: K0, Q0j0, V, then attention interleaved with the
        # remaining QK blocks; phase3 at the end
        for j in range(NB):
            qk_block(1, 0, j)
        qk_block(0, 0, 0)
        for j in range(NPC):
            v_chunk(j)
        if phases >= 2:
            for i in range(NCH):
                for j in range(NB):
                    if not (i == 0 and j == 0):
                        qk_block(0, i, j)
                    attention_pair_block(i, j)
                    if i + 1 < NCH:
                        qk_block(1, i + 1, j)
        else:
            for j in range(1, NB):
                qk_block(0, 0, j)
            for i in range(1, NCH):
                for j in range(NB):
                    qk_block(1, i, j)
                    qk_block(0, i, j)
        if phases >= 3:
            for jj in range(NPC):
                phase3_rows(jj)
    elif drive_mode == "H":
        # nh0-first: emit only work whose inputs live in the first n-half
        # before the first attention block, so exp starts while the second
        # half of x is still being cast/transposed.
        NBH = max(1, NB // 2)      # n_q blocks per half
        NPH = NPC // 2             # V chunks per half
        for j in range(NBH):
            qk_block(1, 0, j)
        qk_block(0, 0, 0)
        for j in range(NPH):
            v_chunk(j)
        if phases >= 2:
            attention_pair_block(0, 0)
            for j in range(NBH, NB):
                qk_block(1, 0, j)
            for j in range(NPH, NPC):
                v_chunk(j)
            for i in range(NCH):
                for j in range(NB):
                    if not (i == 0 and j == 0):
                        qk_block(0, i, j)
                        attention_pair_block(i, j)
                    if i + 1 < NCH:
                        qk_block(1, i + 1, j)
        else:
            for j in range(NBH, NB):
                qk_block(1, 0, j)
            for j in range(NPH, NPC):
                v_chunk(j)
            for j in range(1, NB):
                qk_block(0, 0, j)
            for i in range(1, NCH):
                for j in range(NB):
                    qk_block(1, i, j)
                    qk_block(0, i, j)
        if phases >= 3:
            for jj in range(NPC):
                phase3_rows(jj)
    elif drive_mode == "G":
        # E + phase3 interleaved with the final attention blocks only
        for j in range(NB):
            qk_block(1, 0, j)
        qk_block(0, 0, 0)
        for j in range(NPC):
            v_chunk(j)
        if phases >= 2:
            for j in range(NB):
                if j > 0:
                    qk_block(0, 0, j)
                attention_pair_block(0, j)
                qk_block(1, 1, j)
            for j in range(NB):
                qk_block(0, 1, j)
            for j in range(NB):
                attention_pair_block(1, j)
                if phases >= 3:
                    for jj in range(j * NPB, (j + 1) * NPB):
                        phase3_rows(jj)
    else:  # C: K-first interleaved (previous)
        for i in range(NCH):
            for j in range(NB):
                qk_block(1, i, j)
            qk_block(0, i, 0)
            if i == 0:
                for j in range(NPC):
                    v_chunk(j)
            if phases < 2:
                for j in range(1, NB):
                    qk_block(0, i, j)
                continue
            for j in range(NB):
                if j > 0:
                    qk_block(0, i, j)
                attention_pair_block(i, j)
                if phases >= 3 and i == LH:
                    for jj in range(j * NPB, (j + 1) * NPB):
                        phase3_rows(jj)


def _split_perm(D):
    return np.concatenate([np.arange(0, D, 2), np.arange(1, D, 2)])


def _prep_core_inputs(x, freqs_cis, w_qkv, w_proj, b, heads):
    perm = _split_perm(D)
    qrows, krows = [], []
    for h in heads:
        qrows.append(w_qkv[h * D:(h + 1) * D][perm])
        krows.append(w_qkv[C + h * D:C + (h + 1) * D][perm])
    vrows = [w_qkv[2 * C + h * D:2 * C + (h + 1) * D] for h in heads]
    wqk = np.concatenate(qrows + krows, axis=0)
    wv = np.concatenate(vrows, axis=0)
    hcols = np.concatenate([np.arange(h * D, (h + 1) * D) for h in heads])
    import ml_dtypes
    bf16 = ml_dtypes.bfloat16
    return {
        "xb": np.ascontiguousarray(x[b]).astype(np.float32),
        "wqkT": np.ascontiguousarray(wqk.T).astype(bf16),
        "wvT": np.ascontiguousarray(wv.T).astype(bf16),
        "wpT": np.ascontiguousarray(w_proj[:, hcols].T).astype(bf16),
        "cosT": np.ascontiguousarray(freqs_cis[:, :, 0].T).astype(bf16),
        "sinT": np.ascontiguousarray(
            np.concatenate([freqs_cis[:, :, 1].T, -freqs_cis[:, :, 1].T], axis=0)
        ).astype(bf16),
    }


_CACHE = {}


def _get_compiled():
    if "nc" not in _CACHE:
        nc = bacc.Bacc("TRN2", target_bir_lowering=False, debug=False)
        with tile.TileContext(nc) as tc:
            with ExitStack() as ctx:
                build_attn_kernel(nc, tc, ctx, N=N, C=C, HPC=HPC, D=D, NQ_BLK=512)
        nc.compile()
        _CACHE["nc"] = nc
    return _CACHE["nc"]


def make_in_maps(x, freqs_cis, w_qkv, w_proj):
    x = np.asarray(x, dtype=np.float32)
    freqs_cis = np.asarray(freqs_cis, dtype=np.float32)
    w_qkv = np.asarray(w_qkv, dtype=np.float32)
    w_proj = np.asarray(w_proj, dtype=np.float32)
    in_maps = []
    for c in range(N_CORES):
        b = c // CORES_PER_BATCH
        hg = c % CORES_PER_BATCH
        heads = list(range(hg * HPC, (hg + 1) * HPC))
        in_maps.append(_prep_core_inputs(x, freqs_cis, w_qkv, w_proj, b, heads))
    return in_maps


def gather_output(results, b_proj):
    out = np.zeros((B, N, C), dtype=np.float32)
    for c in range(N_CORES):
        out[c // CORES_PER_BATCH] += results[c]["y"]
    out += np.asarray(b_proj, dtype=np.float32)[None, None, :]
    return out


def kernel(x, freqs_cis, w_qkv, w_proj, b_proj):
    nc = _get_compiled()
    in_maps = make_in_maps(x, freqs_cis, w_qkv, w_proj)
    res = run_bass_kernel_spmd(nc, in_maps, core_ids=list(range(N_CORES)))
    return gather_output(res.results, b_proj)

